# revision 4
# baseline (speedup 1.0000x reference)
"""Trainium2 Bass kernel for FFN (additive) attention.

Reference computation (B=16, S=8192, D=512, H=512):
    q_h = query @ Wq.T + bq                      # (B, H)
    k_h = einsum("bsd,hd->bsh", key, Wk) + bk    # (B, S, H)
    attn_weight = einsum("bsh,h->bs", q_h[:,None,:] + k_h, w_out)
    attn_weight = where(mask, -1e18, attn_weight)
    attn_prob = softmax(attn_weight, -1)
    attn = einsum("bsh,bs->bh", k_h, attn_prob)
    return attn, attn_weight

Key identity: with v = Wk.T @ w_out (D,) and qs[b] = w_out . (Wq@q[b] + bq + bk),
    attn_weight[b,s] = v . key[b,s] + qs[b]
and softmax is invariant to the per-row constant qs[b], while
    attn[b] = Wk @ (sum_s p[b,s] key[b,s]) + bk.
So the device only streams `key` (the 256MB input) once per element:
    pass 1: raw[b,s] = v . key[b,s]   (needs key transposed -> PE transpose)
    e = exp(raw + mask*(-1e18))       (no max subtraction: |raw| <~ 45)
    pass 2: ctx[b,d] = sum_s e[b,s] key[b,s,d]
Host finishes with the tiny (D,H) projections in float64.

float32r everywhere a matmul consumes data: full-rate PE fp32 path.
Sharding: data-parallel over batch, 2 batches per core on 8 cores.
"""

from contextlib import ExitStack

import numpy as np

import concourse.bacc as bacc
import concourse.tile as tile
import concourse.mybir as mybir
from concourse.bass_utils import run_bass_kernel_spmd

B, S, D, H = 16, 8192, 512, 512
NEG_INF = -1e18
NCORES = 8
BPC = B // NCORES          # batches per core = 2
NCHUNK = 8                 # chunks per batch
TPC = 8                    # s-tiles (128) per chunk
NROW = S // 512            # score rows per batch = 16 (one per 512 seq)

F32 = mybir.dt.float32
F32R = mybir.dt.float32r
U8 = mybir.dt.uint8


def build_program():
    """Build the per-core SPMD Bass program. Same program on all 8 cores."""
    nc = bacc.Bacc("TRN2", target_bir_lowering=False, debug=False)

    key_d = nc.dram_tensor("key", [BPC, S, D], F32R, kind="ExternalInput")
    mask_d = nc.dram_tensor("mask", [BPC, NROW, 512], U8, kind="ExternalInput")
    # vs[j, p, :] = [0, v[128*j + p], 0]; lhsT slice [:, j, 1-r:3-r] puts v in
    # output row r of a [2, 512] score tile.
    vs_d = nc.dram_tensor("vs", [4, 128, 3], F32R, kind="ExternalInput")
    id_d = nc.dram_tensor("ident", [128, 128], F32R, kind="ExternalInput")

    s_out = nc.dram_tensor("s_out", [BPC, NROW, 512], F32, kind="ExternalOutput")
    e_out = nc.dram_tensor("e_out", [BPC, NROW, 512], F32R, kind="ExternalOutput")
    ctx_out = nc.dram_tensor("ctx_out", [BPC, 1, D], F32, kind="ExternalOutput")

    # key[b] viewed as [p=128, t=64, d=512]: s = 128*t + p
    key_v = key_d.ap().rearrange("b (t p) d -> b p t d", p=128)

    with tile.TileContext(nc) as tc, ExitStack() as es:
        consts = es.enter_context(tc.tile_pool(name="consts", bufs=1))
        key_pool = es.enter_context(tc.tile_pool(name="key", bufs=6))
        kt_pool = es.enter_context(tc.tile_pool(name="kt", bufs=12))
        m_pool = es.enter_context(tc.tile_pool(name="m", bufs=3))
        mb_pool = es.enter_context(tc.tile_pool(name="mb", bufs=3))
        sm_pool = es.enter_context(tc.tile_pool(name="sm", bufs=4))
        e_pool = es.enter_context(tc.tile_pool(name="e", bufs=4))
        ecol_pool = es.enter_context(tc.tile_pool(name="ecol", bufs=3))
        ctx_sb_pool = es.enter_context(tc.tile_pool(name="ctxsb", bufs=2))

        tp_ps = es.enter_context(tc.tile_pool(name="tp_ps", bufs=4, space="PSUM"))
        sc_ps_pool = es.enter_context(tc.tile_pool(name="sc_ps", bufs=2, space="PSUM"))
        ec_ps_pool = es.enter_context(tc.tile_pool(name="ec_ps", bufs=1, space="PSUM"))
        ctx_ps_pool = es.enter_context(tc.tile_pool(name="ctx_ps", bufs=1, space="PSUM"))

        ident = consts.tile([128, 128], F32R)
        nc.sync.dma_start(out=ident, in_=id_d.ap())
        vs_sb = consts.tile([128, 4, 3], F32R)
        nc.sync.dma_start(out=vs_sb, in_=vs_d.ap().rearrange("j p c -> p j c"))

        for b in range(BPC):
            ctx_ps = ctx_ps_pool.tile([1, D], F32)
            for c in range(NCHUNK):
                key_t = key_pool.tile([128, TPC, D], F32R)
                nc.sync.dma_start(
                    out=key_t,
                    in_=key_v[b, :, c * TPC:(c + 1) * TPC, :],
                )
                m_u8 = m_pool.tile([2, 512], U8)
                nc.sync.dma_start(
                    out=m_u8, in_=mask_d.ap()[b, 2 * c:2 * c + 2, :]
                )
                mb = mb_pool.tile([2, 512], F32)
                nc.vector.tensor_scalar(
                    out=mb, in0=m_u8, scalar1=NEG_INF, scalar2=None,
                    op0=mybir.AluOpType.mult,
                )

                sc_ps = sc_ps_pool.tile([2, 512], F32)
                for r in range(2):
                    for j in range(4):
                        tp = tp_ps.tile([128, 4, 128], F32R)
                        for tg in range(4):
                            tl = 4 * r + tg
                            nc.tensor.transpose(
                                tp[:, tg, :],
                                key_t[:, tl, j * 128:(j + 1) * 128],
                                ident[:, :],
                            )
                        kt = kt_pool.tile([128, 4, 128], F32R)
                        nc.scalar.copy(out=kt, in_=tp)
                        nc.tensor.matmul(
                            sc_ps[:, :],
                            lhsT=vs_sb[:, j, 1 - r:3 - r],
                            rhs=kt[:, :, :],
                            start=(r == 0 and j == 0),
                            stop=(r == 1 and j == 3),
                        )

                sm = sm_pool.tile([2, 512], F32)
                nc.vector.tensor_tensor(
                    out=sm, in0=sc_ps, in1=mb, op=mybir.AluOpType.add
                )
                nc.sync.dma_start(
                    out=s_out.ap()[b, 2 * c:2 * c + 2, :], in_=sm
                )
                e = e_pool.tile([2, 512], F32R)
                nc.scalar.activation(
                    out=e, in_=sm, func=mybir.ActivationFunctionType.Exp
                )
                nc.sync.dma_start(
                    out=e_out.ap()[b, 2 * c:2 * c + 2, :], in_=e
                )

                ec_ps = ec_ps_pool.tile([128, 4, 2], F32R)
                for cc in range(4):
                    nc.tensor.transpose(
                        ec_ps[:, cc, :],
                        e[:, cc * 128:(cc + 1) * 128],
                        ident[0:2, 0:2],
                    )
                ecol = ecol_pool.tile([128, 4, 2], F32R)
                nc.scalar.copy(out=ecol, in_=ec_ps)

                for tl in range(TPC):
                    r, cc = tl // 4, tl % 4
                    nc.tensor.matmul(
                        ctx_ps[:, :],
                        lhsT=ecol[:, cc, r:r + 1],
                        rhs=key_t[:, tl, :],
                        start=(c == 0 and tl == 0),
                        stop=(c == NCHUNK - 1 and tl == TPC - 1),
                    )

            ctx_sb = ctx_sb_pool.tile([1, D], F32)
            nc.scalar.copy(out=ctx_sb, in_=ctx_ps)
            nc.sync.dma_start(out=ctx_out.ap()[b, :, :], in_=ctx_sb)

    nc.compile()
    return nc


def host_prep(query, key, mask, Wq, bq, Wk, bk, w_out):
    """Host-side precompute; returns (in_maps, qs)."""
    w64 = w_out.astype(np.float64)
    v = (Wk.astype(np.float64).T @ w64).astype(np.float32)          # (D,)
    qs = (query.astype(np.float64) @ Wq.astype(np.float64).T
          + bq.astype(np.float64) + bk.astype(np.float64)) @ w64     # (B,)

    vs = np.zeros((4, 128, 3), dtype=np.float32)
    for j in range(4):
        vs[j, :, 1] = v[128 * j:128 * (j + 1)]

    ident = np.eye(128, dtype=np.float32)

    mask_u8 = np.ascontiguousarray(
        mask.astype(np.uint8).reshape(B, NROW, 512)
    )
    key_f32 = np.ascontiguousarray(key.astype(np.float32))

    in_maps = []
    for c in range(NCORES):
        lo = c * BPC
        in_maps.append({
            "key": key_f32[lo:lo + BPC],
            "mask": mask_u8[lo:lo + BPC],
            "vs": vs,
            "ident": ident,
        })
    return in_maps, qs


def host_finish(results, qs, Wk, bk):
    """Gather per-core outputs into (attn, attn_weight)."""
    attn_weight = np.empty((B, S), dtype=np.float32)
    attn = np.empty((B, H), dtype=np.float32)
    Wk64 = Wk.astype(np.float64)
    bk64 = bk.astype(np.float64)
    for c in range(NCORES):
        s_o = results[c]["s_out"].reshape(BPC, S)
        e_o = results[c]["e_out"].reshape(BPC, S).astype(np.float64)
        ctx = results[c]["ctx_out"].reshape(BPC, D).astype(np.float64)
        for i in range(BPC):
            b = c * BPC + i
            attn_weight[b] = s_o[i] + np.float32(qs[b])
            Z = e_o[i].sum()
            attn[b] = (Wk64 @ (ctx[i] / Z) + bk64).astype(np.float32)
    return attn, attn_weight


_NC_CACHE = []


def kernel(query, key, mask, Wq, bq, Wk, bk, w_out):
    query = np.asarray(query)
    key = np.asarray(key)
    mask = np.asarray(mask)
    Wq = np.asarray(Wq)
    bq = np.asarray(bq)
    Wk = np.asarray(Wk)
    bk = np.asarray(bk)
    w_out = np.asarray(w_out)

    if not _NC_CACHE:
        _NC_CACHE.append(build_program())
    nc = _NC_CACHE[0]

    in_maps, qs = host_prep(query, key, mask, Wq, bq, Wk, bk, w_out)
    res = run_bass_kernel_spmd(nc, in_maps, list(range(NCORES)))
    return host_finish(res.results, qs, Wk, bk)


# revision 7
# speedup vs baseline: 1.3227x; 1.3227x over previous
"""Trainium2 Bass kernel for FFN (additive) attention.

Reference computation (B=16, S=8192, D=512, H=512):
    q_h = query @ Wq.T + bq                      # (B, H)
    k_h = einsum("bsd,hd->bsh", key, Wk) + bk    # (B, S, H)
    attn_weight = einsum("bsh,h->bs", q_h[:,None,:] + k_h, w_out)
    attn_weight = where(mask, -1e18, attn_weight)
    attn_prob = softmax(attn_weight, -1)
    attn = einsum("bsh,bs->bh", k_h, attn_prob)
    return attn, attn_weight

Key identity: with v = Wk.T @ w_out (D,) and qs[b] = w_out . (Wq@q[b] + bq + bk),
    attn_weight[b,s] = v . key[b,s] + qs[b]
and softmax is invariant to the per-row constant qs[b], while
    attn[b] = Wk @ (sum_s p[b,s] key[b,s]) + bk.
So the device streams `key` (the 256MB input) exactly once:
    pass 1 (DVE): raw[b,s] = v . key[b,s]  via fused tensor_tensor_reduce
                  on the natural [s-partition, d-free] layout
    e = exp(raw + mask*(-1e18))  on ACT   (no max subtraction: |raw| <~ 45)
    pass 2 (PE):  ctx[b,d] = sum_s e[b,s] key[b,s,d]  as fp32r matmuls with
                  e as the 1-column stationary operand
Host finishes with the tiny (D,H) projections in float64.

Sharding: data-parallel over batch, 2 batches per core on 8 cores.
Device layouts use s = 128*t + p (p = partition, t = 0..63 column).
"""

from contextlib import ExitStack

import numpy as np

import concourse.bass as bass
import concourse.bacc as bacc
import concourse.tile as tile
import concourse.mybir as mybir
from concourse.bass_utils import run_bass_kernel_spmd

B, S, D, H = 16, 8192, 512, 512
NEG_INF = -1e18
NCORES = 8
BPC = B // NCORES          # batches per core = 2
NCHUNK = 8                 # chunks per batch
TPC = 8                    # s-tiles (128 positions) per chunk
NT = S // 128              # s-tiles per batch = 64

F32 = mybir.dt.float32
F32R = mybir.dt.float32r
U8 = mybir.dt.uint8


def build_program():
    """Build the per-core SPMD Bass program. Same program on all 8 cores."""
    nc = bacc.Bacc("TRN2", target_bir_lowering=False, debug=False)

    key_d = nc.dram_tensor("key", [BPC, S, D], F32R, kind="ExternalInput")
    # mbias[b, p, t] = -1e18 * mask[b, 128*t + p]  (host precomputed)
    mb_d = nc.dram_tensor("mbias", [BPC, 128, NT], F32, kind="ExternalInput")
    # v replicated across partitions
    vrep_d = nc.dram_tensor("vrep", [128, D], F32, kind="ExternalInput")

    # s_out[b, p, t] = attn_weight[b, 128*t + p] - qs[b]
    s_out = nc.dram_tensor("s_out", [BPC, 128, NT], F32, kind="ExternalOutput")
    e_out = nc.dram_tensor("e_out", [BPC, 128, NT], F32R, kind="ExternalOutput")
    ctx_out = nc.dram_tensor("ctx_out", [BPC, 1, D], F32, kind="ExternalOutput")

    # key[b] viewed as [p=128, t=64, d=512]: s = 128*t + p
    key_v = key_d.ap().rearrange("b (t p) d -> b p t d", p=128)

    with tile.TileContext(nc) as tc, ExitStack() as es:
        consts = es.enter_context(tc.tile_pool(name="consts", bufs=1))
        key_pool = es.enter_context(tc.tile_pool(name="key", bufs=6))
        scr_pool = es.enter_context(tc.tile_pool(name="scr", bufs=2))
        mb_pool = es.enter_context(tc.tile_pool(name="mb", bufs=2))
        sraw_pool = es.enter_context(tc.tile_pool(name="sraw", bufs=2))
        sm_pool = es.enter_context(tc.tile_pool(name="sm", bufs=2))
        e_pool = es.enter_context(tc.tile_pool(name="e", bufs=2))
        ctx_sb_pool = es.enter_context(tc.tile_pool(name="ctxsb", bufs=2))
        ctx_ps_pool = es.enter_context(tc.tile_pool(name="ctx_ps", bufs=1, space="PSUM"))

        vrep = consts.tile([128, D], F32)
        nc.sync.dma_start(out=vrep, in_=vrep_d.ap())

        for b in range(BPC):
            ctx_ps = ctx_ps_pool.tile([1, D], F32)
            mb_b = mb_pool.tile([128, NT], F32)
            nc.sync.dma_start(out=mb_b, in_=mb_d.ap()[b])
            sraw = sraw_pool.tile([128, NT], F32)
            sm_b = sm_pool.tile([128, NT], F32)
            e_b = e_pool.tile([128, NT], F32R)

            for c in range(NCHUNK):
                key_t = key_pool.tile([128, TPC, D], F32R)
                nc.sync.dma_start(
                    out=key_t,
                    in_=key_v[b, :, c * TPC:(c + 1) * TPC, :],
                )
                scratch = scr_pool.tile([128, TPC, D], F32)
                vr = vrep[:, :]
                vrep_b = bass.AP(
                    tensor=vr.tensor, offset=vr.offset,
                    ap=[vr.ap[0], [0, TPC], vr.ap[1]],
                )
                nc.vector.tensor_tensor(
                    out=scratch,
                    in0=key_t[:, :, :].bitcast(F32),
                    in1=vrep_b,
                    op=mybir.AluOpType.mult,
                )
                for tl in range(TPC):
                    t = c * TPC + tl
                    nc.scalar.activation(
                        out=scratch[:, tl, :],
                        in_=scratch[:, tl, :],
                        func=mybir.ActivationFunctionType.Copy,
                        accum_out=sraw[:, t:t + 1],
                    )
                cs = slice(c * TPC, (c + 1) * TPC)
                nc.vector.tensor_tensor(
                    out=sm_b[:, cs], in0=sraw[:, cs], in1=mb_b[:, cs],
                    op=mybir.AluOpType.add,
                )
                nc.scalar.activation(
                    out=e_b[:, cs], in_=sm_b[:, cs],
                    func=mybir.ActivationFunctionType.Exp,
                )
                for tl in range(TPC):
                    t = c * TPC + tl
                    nc.tensor.matmul(
                        ctx_ps[:, :],
                        lhsT=e_b[:, t:t + 1],
                        rhs=key_t[:, tl, :],
                        start=(c == 0 and tl == 0),
                        stop=(c == NCHUNK - 1 and tl == TPC - 1),
                    )

            nc.sync.dma_start(out=s_out.ap()[b], in_=sm_b)
            nc.sync.dma_start(out=e_out.ap()[b], in_=e_b)
            ctx_sb = ctx_sb_pool.tile([1, D], F32)
            nc.scalar.copy(out=ctx_sb, in_=ctx_ps)
            nc.sync.dma_start(out=ctx_out.ap()[b, :, :], in_=ctx_sb)

    nc.compile()
    return nc


def host_prep(query, key, mask, Wq, bq, Wk, bk, w_out):
    """Host-side precompute; returns (in_maps, qs)."""
    w64 = w_out.astype(np.float64)
    v = (Wk.astype(np.float64).T @ w64).astype(np.float32)          # (D,)
    qs = (query.astype(np.float64) @ Wq.astype(np.float64).T
          + bq.astype(np.float64) + bk.astype(np.float64)) @ w64     # (B,)

    vrep = np.ascontiguousarray(np.broadcast_to(v, (128, D)))

    # mbias[b, p, t] = -1e18 * mask[b, 128*t + p]
    m = mask.reshape(B, NT, 128).transpose(0, 2, 1)   # (B, 128, NT) bool
    mbias = np.ascontiguousarray(
        np.where(m, np.float32(NEG_INF), np.float32(0.0)).astype(np.float32)
    )
    key_f32 = np.ascontiguousarray(key.astype(np.float32))

    in_maps = []
    for c in range(NCORES):
        lo = c * BPC
        in_maps.append({
            "key": key_f32[lo:lo + BPC],
            "mbias": mbias[lo:lo + BPC],
            "vrep": vrep,
        })
    return in_maps, qs


def host_finish(results, qs, Wk, bk):
    """Gather per-core outputs into (attn, attn_weight)."""
    attn_weight = np.empty((B, S), dtype=np.float32)
    attn = np.empty((B, H), dtype=np.float32)
    Wk64 = Wk.astype(np.float64)
    bk64 = bk.astype(np.float64)
    for c in range(NCORES):
        # device layout [p, t] -> s = 128*t + p
        s_o = results[c]["s_out"].transpose(0, 2, 1).reshape(BPC, S)
        e_o = results[c]["e_out"].transpose(0, 2, 1).reshape(BPC, S)
        ctx = results[c]["ctx_out"].reshape(BPC, D).astype(np.float64)
        for i in range(BPC):
            b = c * BPC + i
            attn_weight[b] = s_o[i] + np.float32(qs[b])
            Z = e_o[i].astype(np.float64).sum()
            attn[b] = (Wk64 @ (ctx[i] / Z) + bk64).astype(np.float32)
    return attn, attn_weight


_NC_CACHE = []


def kernel(query, key, mask, Wq, bq, Wk, bk, w_out):
    query = np.asarray(query)
    key = np.asarray(key)
    mask = np.asarray(mask)
    Wq = np.asarray(Wq)
    bq = np.asarray(bq)
    Wk = np.asarray(Wk)
    bk = np.asarray(bk)
    w_out = np.asarray(w_out)

    if not _NC_CACHE:
        _NC_CACHE.append(build_program())
    nc = _NC_CACHE[0]

    in_maps, qs = host_prep(query, key, mask, Wq, bq, Wk, bk, w_out)
    res = run_bass_kernel_spmd(nc, in_maps, list(range(NCORES)))
    return host_finish(res.results, qs, Wk, bk)
